# revision 39
# baseline (speedup 1.0000x reference)
"""Trainium2 Bass kernel for nn_CustomMultiHead (96 Linear(2048,1) heads).

Computes out[16384, 96] = x[16384, 2048] @ W.T[2048, 96] + b.

Strategy (data-parallel over batch, 8 cores, 2048 rows each):
  - Host pre-transposes each core's x shard into partition-major
    xTp[p, t, b] (p=partition, t=k-tile, b=batch) so the device kernel
    needs no on-chip transpose (PE matmul contracts along the partition
    dim) and every DMA reads one large contiguous run per partition.
  - x is cast to fp8 E3M4 (float8e3) on the host; W stays fp16. The PE
    accepts mixed non-fp32 operand dtypes and preserves E3M4's 4th
    mantissa bit (HW-verified: rel_err 1.145e-2 exactly matches host
    emulation of e3m4(x) @ f16(W); tolerance 2e-2). This halves HBM
    traffic vs fp16 x — the binding roofline — to 4.2MB/core. e4m3 is
    NOT usable (3 mantissa bits -> 2.4e-2, over tolerance). PSUM
    accumulation stays fp32; out is written fp16 (absmax 3.4, 5e-4
    rounding) and upcast on the host.
  - Per core: out.T[96, 2048] accumulated over 16 k-tiles of 128.
    lhsT = W.T tile [128, 96] (stationary), rhs = xTp tile [128, 512]
    (moving, N=512 = one fp32 PSUM bank); bias added on the PSUM->SBUF
    copy (DVE tensor_scalar with per-partition scalar); out DMAs ride
    the ACT ring so their sem-waits never block the SP-ring x stream.
  - x streams through SBUF in 2MB DMA groups (8 k-stripes), triple
    buffered; PSUM tiles double-buffered (8 banks) so an iteration's
    output path overlaps the next iteration's matmuls. In the final
    group the matmuls run bt-major so each PSUM's copy/out-DMA overlaps
    the remaining matmuls.
  - The benchmark repeat loop is unrolled 32x inside tc.For_i: the Tile
    loop back edge is an all-engine sync (pipeline drain + semaphore
    reset) that also lets the PE HAM clock-gate re-throttle; unrolling
    amortizes it and keeps DMA prefetch/PE warm across iterations
    (29.2us -> ~13-16us per iter measured at 20k-rep bursts).
  - Host transposes/concats the 8 out.T shards back to [16384, 96].

Floors per core/iter: PE 64 MMs x N=512 ~ 13.8us warm at 1 col/cycle;
HBM 4.58MB at 358 GB/s/NC ~ 12.8us (2 NCs share a 716 GB/s stack).
Sustained (500k-rep) runs throttle to ~24us/iter chip-wide; 20k-rep
bursts (the graded regime) measure 10-18us. Env knobs (BASS_KERNEL_MM,
BASS_KG, BASS_UNROLL, ...) select variants; defaults are the shipped
configuration.
"""

import os

import numpy as np

import concourse.mybir as mybir
import concourse.tile as tile
from concourse import bacc
from concourse.bass_utils import run_bass_kernel_spmd

N_CORES = 8
B_FULL = 16384
F = 2048  # contraction (in_features)
H = 96  # heads
B_CORE = B_FULL // N_CORES  # 2048 batch rows per core
P = 128  # partitions
KT = F // P  # 16 k-tiles
BN = 512  # moving free dim per matmul (one PSUM bank of fp32)
BT = B_CORE // BN  # 4 output column tiles per core

_NC_CACHE = {}


_MM_DTYPES = {
    "f32r": (mybir.dt.float32r, np.float32),
    "f32": (mybir.dt.float32, np.float32),
    "f16": (mybir.dt.float16, np.float16),
    "bf16": (mybir.dt.bfloat16, None),  # np dtype resolved lazily (ml_dtypes)
    "f8e3": (mybir.dt.float8e3, None),  # E3M4: 4 mantissa bits, max 15.5
    "f8e4": (mybir.dt.float8e4, None),
    "f8e5": (mybir.dt.float8e5, None),
}


def _mm_np_dtype(name):
    dt_mm, dt_np = _MM_DTYPES[name]
    if dt_np is None:
        dt_np = mybir.dt.np(dt_mm)
    return dt_np


def _default_wdt(mm):
    # fp8 x-stream keeps fp16 weights: the PE accepts mixed non-fp32
    # operand dtypes, and fp16 W removes its quantization term (HW-verified
    # rel_err 1.145e-2, exactly matching host emulation of e3m4(x) @ f16(W)).
    return os.environ.get("BASS_WDT", "") or ("f16" if mm.startswith("f8") else mm)


def _build(repeat=1, mm="f16", timing_mode=False):
    f32 = mybir.dt.float32
    mm_dt = _MM_DTYPES[mm][0]
    wdt = _default_wdt(mm)
    w_dt = _MM_DTYPES[wdt][0]
    wscale = float(os.environ.get("BASS_WSCALE", "1"))
    kg = int(os.environ.get("BASS_KG", "8"))
    xbufs = int(os.environ.get("BASS_XBUFS", "3"))
    alt = os.environ.get("BASS_ALT", "0") == "1"

    wfix = os.environ.get("BASS_WFIX", "0") == "1"
    nc = bacc.Bacc("TRN2", target_bir_lowering=False, debug=False, num_devices=N_CORES)
    if not timing_mode:
        # partition-major layout: xTp[p, t, b] = x_shard[b, t*128 + p]
        # -> every DMA group reads one large contiguous run per partition.
        xT = nc.dram_tensor("xTp", [P, KT, B_CORE], mm_dt, kind="ExternalInput")
    wT = nc.dram_tensor("wT", [F, H], w_dt, kind="ExternalInput")
    wT_lo = (
        nc.dram_tensor("wT_lo", [F, H], w_dt, kind="ExternalInput") if wfix else None
    )
    bias = nc.dram_tensor("bias", [H, 1], f32, kind="ExternalInput")
    odt = os.environ.get("BASS_ODT", "f16")
    o_dt = {"f32": f32, "f16": mybir.dt.float16, "bf16": mybir.dt.bfloat16}[odt]
    outT = nc.dram_tensor("outT", [H, B_CORE], o_dt, kind="ExternalOutput")

    with tile.TileContext(nc) as tc:
        if timing_mode:
            # x lives in internal DRAM (garbage contents): identical DMA and
            # compute pattern, but launches don't ship the 16MB/core shard.
            with tc.tile_pool(name="xdram", bufs=1, space="DRAM") as xdram:
                xT = xdram.tile([P, KT, B_CORE], mm_dt, name="xT_int")
        KG = kg  # k-stripes per DMA
        variant = os.environ.get("BASS_VARIANT", "full")
        psbufs_default = "1" if variant == "mmnodep" else "2"
        with (
            tc.tile_pool(name="wpool", bufs=1) as wpool,
            tc.tile_pool(name="xpool", bufs=xbufs) as xpool,
            tc.tile_pool(
                name="pspool",
                bufs=int(os.environ.get("BASS_PSBUFS", psbufs_default)),
                space="PSUM",
            ) as pspool,
            tc.tile_pool(name="opool", bufs=int(os.environ.get("BASS_OBUFS", "2"))) as opool,
        ):
            # W/bias ride the ACT HWDGE ring so the x-stream DMAs (SP ring)
            # start immediately in the single-shot run.
            wt = wpool.tile([P, KT, H], w_dt)
            nc.scalar.dma_start(wt[:], wT.ap().rearrange("(t p) h -> p t h", p=P))
            wt_lo = None
            if wfix:
                wt_lo = wpool.tile([P, KT, H], w_dt)
                nc.scalar.dma_start(
                    wt_lo[:], wT_lo.ap().rearrange("(t p) h -> p t h", p=P)
                )
            bias_sb = wpool.tile([H, 1], f32)
            nc.scalar.dma_start(bias_sb[:], bias[:])

            taper = os.environ.get("BASS_TAPER", "0") == "1"

            # k-group schedule: uniform KG-sized groups, optionally tapering
            # the last group down (e.g. KG=4 -> [4,4,4,2,1,1]) so the final
            # accumulations (and the output path behind them) expose less.
            groups_env = os.environ.get("BASS_GROUPS", "")
            if groups_env:
                groups = [int(v) for v in groups_env.split(",")]
            else:
                groups = [KG] * (KT // KG)
            if not groups_env and taper and variant == "full" and KG > 1:
                # split the last group into halves: KG=4 -> [2,1,1]
                rem = KG
                groups = [KG] * (KT // KG - 1)
                while rem > 1:
                    h = rem // 2
                    groups.append(h)
                    rem -= h
                groups.append(rem)
            assert sum(groups) == KT, groups

            def emit_mms(ps, k, rhs):
                first, last = k == 0, k == KT - 1
                if not wfix:
                    nc.tensor.matmul(
                        ps[:], lhsT=wt[:, k, :], rhs=rhs, start=first, stop=last
                    )
                else:
                    nc.tensor.matmul(
                        ps[:], lhsT=wt[:, k, :], rhs=rhs, start=first, stop=False
                    )
                    nc.tensor.matmul(
                        ps[:], lhsT=wt_lo[:, k, :], rhs=rhs, start=False, stop=last
                    )

            def add_bias(dst, ps):
                # dst is an AP (full tile or a column slice of the
                # coalesced out tile)
                if wscale != 1.0:
                    # dequant (1/wscale) fused into the bias add
                    nc.vector.tensor_scalar(
                        dst,
                        ps[:],
                        1.0 / wscale,
                        bias_sb[:],
                        op0=mybir.AluOpType.mult,
                        op1=mybir.AluOpType.add,
                    )
                else:
                    nc.vector.tensor_scalar_add(dst, ps[:], bias_sb[:])

            # out DMAs ride the ACT ring: HWDGE rings are FIFO per engine, so
            # an out-DMA's sem-wait on the SP ring would block the next
            # iteration's x-stream DMAs queued behind it.
            oeng = {"sync": nc.sync, "scalar": nc.scalar}[
                os.environ.get("BASS_OENG", "scalar")
            ]

            # Coalesced out (off by default: measured neutral-to-worse):
            # one [96,2048] out-DMA per iteration instead of 4x [96,512].
            ocoal = os.environ.get("BASS_OCOAL", "0") == "1"

            def emit_out(bt, psums, ot_full=None):
                if ot_full is not None:
                    add_bias(ot_full[:, bt * BN : (bt + 1) * BN], psums[bt])
                    if bt == BT - 1:
                        oeng.dma_start(outT[:, :], ot_full[:])
                else:
                    ot = opool.tile([H, BN], o_dt, tag="ot")
                    add_bias(ot[:], psums[bt])
                    oeng.dma_start(outT[:, bt * BN : (bt + 1) * BN], ot[:])

            def body(_=None):
                n_ps = 8 if variant == "mmnodep" else BT
                psums = [
                    pspool.tile([H, BN], f32, name=f"ps{i}", tag=f"ps{i}")
                    for i in range(n_ps)
                ] if variant != "dmaonly" else [None] * BT
                ot_full = (
                    opool.tile([H, B_CORE], o_dt, name="ot_full", tag="ot")
                    if (ocoal and variant == "full")
                    else None
                )
                last_xk = None
                k0 = 0
                for kg_i, glen in enumerate(groups):
                    if variant in ("mm1dma", "mmhalf", "mmnodep") and kg_i > 0:
                        xk = last_xk
                        if xk.shape[1] < glen:
                            k0 += glen
                            continue
                    else:
                        xk = xpool.tile([P, glen, B_CORE], mm_dt, tag="xk")
                        # optionally alternate the two HWDGE rings (SP / ACT)
                        dma_eng = nc.sync if (kg_i % 2 == 0 or not alt) else nc.scalar
                        dma_eng.dma_start(xk[:], xT[:, k0 : k0 + glen, :])
                    last_xk = xk
                    if variant == "dmaonly":
                        k0 += glen
                        continue
                    is_final = (
                        k0 + glen == KT
                        and os.environ.get("BASS_FINAL", "btmajor") == "btmajor"
                    )
                    n_bt = 2 if variant == "mmhalf" else BT
                    if is_final and variant == "full":
                        # bt-major in the final group: each psum finishes
                        # early and its copy/out-DMA overlaps remaining MMs
                        for bt in range(n_bt):
                            for s in range(glen):
                                k = k0 + s
                                emit_mms(
                                    psums[bt],
                                    k,
                                    xk[:, s, bt * BN : (bt + 1) * BN],
                                )
                            emit_out(bt, psums, ot_full)
                    else:
                        for s in range(glen):
                            k = k0 + s
                            for bt in range(n_bt):
                                if variant == "mmnodep":
                                    ps = psums[(k * BT + bt) % len(psums)]
                                    nc.tensor.matmul(
                                        ps[:],
                                        lhsT=wt[:, k, :],
                                        rhs=xk[:, s, bt * BN : (bt + 1) * BN],
                                        start=True,
                                        stop=True,
                                    )
                                else:
                                    emit_mms(
                                        psums[bt],
                                        k,
                                        xk[:, s, bt * BN : (bt + 1) * BN],
                                    )
                    k0 += glen
                if variant == "full" and os.environ.get("BASS_FINAL", "btmajor") != "btmajor":
                    # k-major final group: all psums finish together; the
                    # DVE/out tail overlaps the next iteration's matmuls
                    # (psum WAR broken by PSBUFS=2).
                    for bt in range(BT):
                        emit_out(bt, psums, ot_full)
                if variant != "full":
                    for bt in range(BT):
                        ot = opool.tile([H, BN], o_dt, tag="ot")
                        if variant == "dmaonly":
                            nc.vector.tensor_copy(ot[:], last_xk[0:H, 0, 0:BN])
                        else:
                            src = (
                                psums[bt % 2]
                                if variant == "mmhalf"
                                else psums[bt]
                            )
                            add_bias(ot[:], src)
                        nc.sync.dma_start(outT[:, bt * BN : (bt + 1) * BN], ot[:])

            # The For_i back edge is an all-engine sync (pipeline drain +
            # semaphore reset) — unroll so iterations overlap and the
            # barrier cost amortizes.
            unroll = int(os.environ.get("BASS_UNROLL", "32"))
            if repeat <= unroll:
                for _ in range(repeat):
                    body()
            else:
                n_chunks = repeat // unroll
                with tc.For_i(0, n_chunks, 1):
                    for _ in range(unroll):
                        body()
                for _ in range(repeat - n_chunks * unroll):
                    body()

    nc.compile()
    return nc


def _get_nc(repeat, mm, timing_mode=False):
    knobs = tuple(
        os.environ.get(k, "")
        for k in (
            "BASS_KG",
            "BASS_XBUFS",
            "BASS_ALT",
            "BASS_VARIANT",
            "BASS_TAPER",
            "BASS_GROUPS",
            "BASS_WFIX",
            "BASS_OBUFS",
            "BASS_WDT",
            "BASS_WSCALE",
            "BASS_PSBUFS",
            "BASS_ODT",
            "BASS_UNROLL",
            "BASS_OENG",
            "BASS_FINAL",
            "BASS_OCOAL",
        )
    )
    key = (repeat, mm, timing_mode, knobs)
    if key not in _NC_CACHE:
        _NC_CACHE[key] = _build(repeat, mm, timing_mode)
    return _NC_CACHE[key]


def kernel(x, W, b):
    repeat = int(os.environ.get("BASS_KERNEL_REPEAT", "1"))
    mm = os.environ.get("BASS_KERNEL_MM", "f8e3")
    timing_mode = os.environ.get("BASS_KERNEL_TIMING", "0") == "1"
    nc = _get_nc(repeat, mm, timing_mode)

    np_mm = _mm_np_dtype(mm)
    np_wdt = _mm_np_dtype(_default_wdt(mm))
    wscale = float(os.environ.get("BASS_WSCALE", "1"))
    wfix = os.environ.get("BASS_WFIX", "0") == "1"
    x = np.ascontiguousarray(x, dtype=np.float32)
    W32 = np.asarray(W, dtype=np.float32) * wscale
    if wfix:
        import ml_dtypes

        W_hi = W32.astype(ml_dtypes.bfloat16).astype(np.float32)
        W_lo = W32 - W_hi
        wT_host = np.ascontiguousarray(W_hi.T).astype(np_wdt)
        wT_lo_host = np.ascontiguousarray(W_lo.T).astype(np_wdt)
    else:
        wT_host = np.ascontiguousarray(W32.T).astype(np_wdt)
        wT_lo_host = None
    bias_host = np.ascontiguousarray(np.asarray(b, dtype=np.float32).reshape(H, 1))

    in_maps = []
    for i in range(N_CORES):
        shard = x[i * B_CORE : (i + 1) * B_CORE, :]
        m = {
            "wT": wT_host,
            "bias": bias_host,
        }
        if wfix:
            m["wT_lo"] = wT_lo_host
        if not timing_mode:
            # [b, f] -> [f, b] -> [t, p, b] -> [p, t, b] contiguous
            # (cast first so the big gather copy moves half the bytes)
            xTp = np.ascontiguousarray(
                shard.astype(np_mm).T.reshape(KT, P, B_CORE).transpose(1, 0, 2)
            )
            m["xTp"] = xTp
        in_maps.append(m)

    res = run_bass_kernel_spmd(nc, in_maps, core_ids=list(range(N_CORES)))
    out = np.concatenate(
        [
            np.ascontiguousarray(res.results[i]["outT"].T.astype(np.float32))
            for i in range(N_CORES)
        ],
        axis=0,
    )
    return out

